# revision 28
# baseline (speedup 1.0000x reference)
"""Multi-headed causal attention on 8 trn2 NeuronCores (Bass/Tile) — v2.

Sharding: tensor-parallel over heads — 2 heads per core, all 4 batches.
v2 redesign vs v1 (463us):
  - bf16 activations/weights end-to-end (host casts embedded/W to bf16):
    halves the 32MB startup DMA and the projection operand traffic.
  - ACT (scalar engine) does ONLY the exp in the attention inner loop;
    causal masking is a precomputed multiplicative bf16 mask applied on
    DVE (was: 1.4us affine_select on GpSimd per diagonal tile).
  - softmax normalization stays on the sender but is restructured:
    DVE dn-copy + fast reciprocal, GpSimd partition_broadcast, fused
    DVE (psum x rb -> bf16) multiply. No ACT involvement.
  - AllToAll split into 8 small per-(batch, seq-half) collectives of
    256KB each, fired mid-attention as soon as their half completes;
    ownership is 128-token stripes per core per batch so every A2A
    carries all 8 destinations. Output projection for each (b,x) block
    is interleaved into the following batch's attention emission.
  - projections of batch b+1 are interleaved m-loop-granular into batch
    b's attention so the PE queue never drains while ACT paces exp.
  - PE warmup matmuls during the initial DMA fill (HAM un-throttle).
  - output-projection bias folded into a K=1 matmul; out rows are
    (batch, half, stripe) blocks reassembled on host.
"""
import sys

sys.path.insert(0, "/opt/trn_rl_repo")

import numpy as np

import concourse.bass as bass
import concourse.tile as tile
from concourse import bacc, mybir
from concourse.bass_utils import run_bass_kernel_spmd

B, S, D, H, HD = 4, 2048, 1024, 16, 64
NC_ = 8          # cores
PH = 2           # heads per core
SC = 512         # s_q chunk
NK = S // 128    # 16 s_k chunks of 128
ND = D // 128    # 8 contraction chunks of 128
DEBUG = False
F32 = mybir.dt.float32
BF16 = mybir.dt.bfloat16
EXP = mybir.ActivationFunctionType.Exp
GE = mybir.AluOpType.is_ge


def build():
    nc = bacc.Bacc("TRN2", target_bir_lowering=False, debug=False, num_devices=NC_)

    emb_t = nc.dram_tensor("embedded_t", [B, D, S], BF16, kind="ExternalInput").ap()
    w_qkv = nc.dram_tensor("w_qkv", [3, ND, 128, 128], BF16, kind="ExternalInput").ap()
    wo_t = nc.dram_tensor("wo_t", [ND, 128, D], BF16, kind="ExternalInput").ap()
    bo_row = nc.dram_tensor("bo_row", [1, D], BF16, kind="ExternalInput").ap()
    out_shard = nc.dram_tensor("out_shard", [1024, D], F32, kind="ExternalOutput").ap()
    dbg = None
    if DEBUG:
        dbg = {
            "qt": nc.dram_tensor("dbg_qt", [128, S], BF16, kind="ExternalOutput").ap(),
            "kt0": nc.dram_tensor("dbg_kt0", [128, S], BF16, kind="ExternalOutput").ap(),
            "v00": nc.dram_tensor("dbg_v00", [128, NK, 128], BF16, kind="ExternalOutput").ap(),
            "ex": nc.dram_tensor("dbg_ex", [128, 2, SC], BF16, kind="ExternalOutput").ap(),
            "cu": nc.dram_tensor("dbg_cu", [65, SC], F32, kind="ExternalOutput").ap(),
            "cn": nc.dram_tensor("dbg_cn", [64, SC], BF16, kind="ExternalOutput").ap(),
            "cat": nc.dram_tensor("dbg_cat", [NC_, 128, 128], BF16, kind="ExternalOutput").ap(),
        }

    with tile.TileContext(nc) as tc:
        _build_body(nc, tc, emb_t, w_qkv, wo_t, bo_row, out_shard, dbg)

    nc.compile()
    return nc


def _build_body(nc, tc, emb_t, w_qkv, wo_t, bo_row, out_shard, dbg=None):
    from contextlib import ExitStack

    ctx = ExitStack()
    with ctx:
        const = ctx.enter_context(tc.tile_pool(name="const", bufs=1))
        # PSUM: scores 2x[128,1024](4 banks) + ctx 2x[128,512](2) +
        # misc 2x[128,512](2) = 8 banks
        ps_sc = ctx.enter_context(tc.tile_pool(name="ps_sc", bufs=2, space="PSUM"))
        ps_ctx = ctx.enter_context(tc.tile_pool(name="ps_ctx", bufs=2, space="PSUM"))
        ps_ms = ctx.enter_context(tc.tile_pool(name="ps_ms", bufs=2, space="PSUM"))
        dram = ctx.enter_context(tc.tile_pool(name="dram", bufs=1, space="DRAM"))

        etp = ctx.enter_context(tc.tile_pool(name="etp", bufs=16))
        qtp = ctx.enter_context(tc.tile_pool(name="qtp", bufs=2))
        ktp = ctx.enter_context(tc.tile_pool(name="ktp", bufs=2))
        vtp = ctx.enter_context(tc.tile_pool(name="vtp", bufs=2))
        vsb = ctx.enter_context(tc.tile_pool(name="vsb", bufs=2))
        exp_p = ctx.enter_context(tc.tile_pool(name="exp_p", bufs=4))
        cn_p = ctx.enter_context(tc.tile_pool(name="cn_p", bufs=4))
        rc_p = ctx.enter_context(tc.tile_pool(name="rc_p", bufs=4))
        rb_p = ctx.enter_context(tc.tile_pool(name="rb_p", bufs=2))
        cat_p = ctx.enter_context(tc.tile_pool(name="cat_p", bufs=16))
        ob_p = ctx.enter_context(tc.tile_pool(name="ob_p", bufs=2))

        # ---- startup DMAs (SP queue): weights, batch-0 activations ----
        wq_all = const.tile([128, 24, 128], BF16, tag="wq_all")
        for p in range(3):
            nc.sync.dma_start(out=wq_all[:, 8 * p:8 * (p + 1), :],
                              in_=bass.AP(
                tensor=w_qkv.tensor, offset=131072 * p,
                ap=[[128, 128], [16384, 8], [1, 128]]))
        wq_sb = [[wq_all[:, 8 * p + c, :] for c in range(ND)] for p in range(3)]

        wot_sb = [const.tile([128, D], BF16, tag=f"wo{c}", name=f"wo{c}")
                  for c in range(ND)]
        for c in range(ND):
            nc.sync.dma_start(out=wot_sb[c][:], in_=wo_t[c])
        bo_sb = const.tile([1, D], BF16, tag="bo1")
        nc.sync.dma_start(out=bo_sb[:], in_=bo_row[:])

        # ---- PE warmup first: its memset must not queue behind the
        # slow gpsimd mask builds below ----
        wu_src = const.tile([128, SC], BF16, tag="wu")
        nc.gpsimd.memset(wu_src[:], 0.0)
        wu_ps = ps_ctx.tile([128, SC], F32, tag="ctx", name="wu_ps")
        for _ in range(22):
            nc.tensor.matmul(wu_ps[:], lhsT=wu_src[:, 0:128], rhs=wu_src[:],
                             start=True, stop=True)

        # ---- constants ----
        ones_bf = const.tile([1, 128], BF16, tag="ones_bf")
        nc.gpsimd.memset(ones_bf[:], 1.0)
        ones_r = const.tile([128, 1], BF16, tag="ones_r")
        nc.gpsimd.memset(ones_r[:], 1.0)

        ident = const.tile([128, 128], BF16, tag="ident")
        nc.gpsimd.memset(ident[:], 1.0)
        nc.gpsimd.affine_select(out=ident[:], in_=ident[:], compare_op=GE,
                                fill=0.0, base=0, pattern=[[-1, 128]],
                                channel_multiplier=1)
        nc.gpsimd.affine_select(out=ident[:], in_=ident[:], compare_op=GE,
                                fill=0.0, base=0, pattern=[[1, 128]],
                                channel_multiplier=-1)

        # causal masks for diagonal tiles: keep col c (mod 512) >= p + 128*mi
        masks = []
        for mi in range(4):
            mk = const.tile([128, 2, SC], BF16, tag=f"mask{mi}", name=f"mask{mi}")
            nc.gpsimd.memset(mk[:], 1.0)
            nc.gpsimd.affine_select(
                out=mk[:], in_=mk[:], compare_op=GE, fill=0.0,
                base=-128 * mi, pattern=[[0, 2], [1, SC]],
                channel_multiplier=-1)
            masks.append(mk)

        # a2a buffers: one pair per (batch, seq-half); slot o = 128-token
        # stripe for core o, rows = this core's 128 ctx dims (normalized)
        a2a_in = [[dram.tile([NC_, 128, 128], BF16, tag=f"a2a_in{b}_{x}",
                             name=f"a2a_in{b}_{x}") for x in range(2)]
                  for b in range(B)]
        a2a_out = [[dram.tile([NC_, 128, 128], BF16, tag=f"a2a_out{b}_{x}",
                              name=f"a2a_out{b}_{x}")
                    for x in range(2)] for b in range(B)]

        def emit_a2a(b, x):
            nc.gpsimd.collective_compute(
                "AllToAll", mybir.AluOpType.bypass,
                replica_groups=[list(range(NC_))],
                ins=[a2a_in[b][x].opt()], outs=[a2a_out[b][x].opt()])

        # ---------- phase builders ----------
        state = {}

        def emit_et_loads(b, split=False):
            et = {c: etp.tile([128, S], BF16, tag="et", name=f"et{b}_{c}")
                  for c in range(ND)}
            if split:
                # quartered, two queues: the first pj-group can start after
                # only the j4=0 column block has landed
                for j4 in range(4):
                    for c in range(ND):
                        eng = nc.sync if (c % 2 == 0) else nc.scalar
                        eng.dma_start(
                            out=et[c][:, SC * j4:SC * (j4 + 1)],
                            in_=emb_t[b, 128 * c:128 * (c + 1),
                                      SC * j4:SC * (j4 + 1)])
            else:
                # ACT queue: keeps SP free for cat loads (out-proj path)
                for c in range(ND):
                    nc.scalar.dma_start(
                        out=et[c][:], in_=emb_t[b, 128 * c:128 * (c + 1), :])
            return et

        def start_proj(b):
            """Allocate batch-b projection outputs + zero-pads; return the
            list of 12 pj-group closures and the finish closure."""
            qt = qtp.tile([128, S], BF16, tag="qt", name=f"qt{b}")
            kt0 = ktp.tile([128, S], BF16, tag="kt0", name=f"kt0_{b}")
            kt1 = ktp.tile([128, S], BF16, tag="kt1", name=f"kt1_{b}")
            nc.vector.memset(kt0[64:128, :], 0.0)
            nc.vector.memset(kt1[0:64, :], 0.0)
            vt = vtp.tile([128, S], BF16, tag="vt", name=f"vt{b}")
            groups = []
            # emission order: vt groups first (transposes need them), then
            # qt/kt for j4 0-1 (needed by next batch's j=0/1), then the rest
            # which fill the inter-batch PE bubble
            order = ([(j4, 2) for j4 in range(4)]
                     + [(j4, p) for j4 in (0, 1) for p in (0, 1)]
                     + [(j4, p) for j4 in (2, 3) for p in (0, 1)])
            for j4, p in order:
                if True:
                    def g(j4=j4, p=p):
                        sl = slice(SC * j4, SC * (j4 + 1))
                        ps = ps_ms.tile([128, SC], F32, tag="ms",
                                        name=f"pj{b}_{j4}_{p}")
                        for c in range(ND):
                            rhs = state[("et", b)][c][:, sl]
                            nc.tensor.matmul(
                                ps[:], lhsT=wq_sb[p][c], rhs=rhs,
                                start=(c == 0), stop=(c == ND - 1))
                        if p == 0:
                            nc.vector.tensor_copy(qt[:, sl], ps[:])
                        elif p == 1:
                            nc.vector.tensor_copy(kt0[0:64, sl], ps[0:64, :])
                            nc.vector.tensor_copy(kt1[64:128, sl], ps[64:128, :])
                        else:
                            nc.scalar.copy(vt[:, sl], ps[:])
                    groups.append(g)

            def finish():
                # V natural layout padded to 128 cols: V | ones | zeros
                v01 = [vsb.tile([128, NK, 128], BF16, tag=f"v{h}",
                                name=f"v{h}_{b}") for h in range(PH)]
                for h in range(PH):
                    nc.vector.memset(v01[h][:, :, 65:128], 0.0)
                for sk in range(NK):
                    pt = ps_ms.tile([128, 128], BF16, tag="ms",
                                    name=f"tr{b}_{sk}")
                    nc.tensor.transpose(pt[:], vt[:, 128 * sk:128 * (sk + 1)],
                                        ident[:])
                    for h in range(PH):
                        nc.vector.tensor_copy(v01[h][:, sk, 0:64],
                                              pt[:, 64 * h:64 * (h + 1)])
                        nc.vector.tensor_copy(v01[h][:, sk, 64:65], ones_r[:])
                state[("v01", b)] = v01

            state[("qt", b)] = qt
            state[("kts", b)] = [kt0, kt1]
            return groups, finish

        def emit_outproj(b, x):
            """cat loads + output projection for this core's 128-token
            stripe of (batch b, half x)."""
            cats = []
            for r in range(NC_):
                ct = cat_p.tile([128, 128], BF16, tag="cat",
                                name=f"cat{b}_{x}_{r}")
                nc.sync.dma_start(out=ct[:], in_=a2a_out[b][x][r])
                if dbg is not None and b == 0 and x == 0:
                    nc.sync.dma_start(out=dbg["cat"][r], in_=ct[:])
                cats.append(ct)
            row0 = 256 * b + 128 * x
            for n in range(2):
                po = ps_ms.tile([128, SC], F32, tag="ms", name=f"po{b}_{x}_{n}")
                for kp in range(ND):
                    nc.tensor.matmul(
                        po[:], lhsT=cats[kp][:],
                        rhs=wot_sb[kp][:, SC * n:SC * (n + 1)],
                        start=(kp == 0), stop=False)
                nc.tensor.matmul(
                    po[:], lhsT=ones_bf[:],
                    rhs=bo_sb[:, SC * n:SC * (n + 1)],
                    start=False, stop=True)
                ob = ob_p.tile([128, SC], F32, tag="ob")
                nc.scalar.copy(ob[:], po[:])
                nc.gpsimd.dma_start(
                    out=out_shard[row0:row0 + 128, SC * n:SC * (n + 1)],
                    in_=ob[:])

        def emit_attention(b, hooks):
            """hooks: dict j -> list of closures emitted after chunk j's
            m-loop (projection groups of b+1, out-proj blocks, ...)."""
            qt = state[("qt", b)]
            kts = state[("kts", b)]
            v01 = state[("v01", b)]
            pj_queue = state.get(("pjq", b + 1), [])
            pj_i = [0]

            def drain_pj(n):
                for _ in range(n):
                    if pj_i[0] < len(pj_queue):
                        pj_queue[pj_i[0]]()
                        pj_i[0] += 1

            # last batch: (0,2,3,1) so the final A2A's transfer overlaps
            # the j=1 m-loop and out-proj(3,1)
            j_order = (0, 2, 3, 1) if b == B - 1 else (0, 1, 2, 3)
            done = set()
            for pos, j in enumerate(j_order):
                mtop = 4 * j + 4
                ctx_ps = [ps_ctx.tile([128, SC], F32, tag="ctx",
                                      name=f"ctx{b}_{j}_{h}")
                          for h in range(PH)]
                PIPE = 1
                exq = []

                def emit_scores(m, j=j, ctx_ps=ctx_ps):
                    c0 = max(0, 128 * m - SC * j)
                    psc = ps_sc.tile([128, 2, SC], F32, tag="sc",
                                     name=f"sc{b}_{j}_{m}")
                    for h in range(PH):
                        nc.tensor.matmul(
                            psc[:, h, c0:],
                            lhsT=kts[h][:, 128 * m:128 * (m + 1)],
                            rhs=qt[:, SC * j + c0:SC * (j + 1)],
                            start=True, stop=True)
                    ex = exp_p.tile([128, 2, SC], BF16, tag="ex",
                                    name=f"ex{b}_{j}_{m}")
                    if c0 > 0:
                        # zero the never-exp'd leading zones so the mask
                        # multiply can't hit stale inf/NaN
                        nc.vector.memset(ex[:, :, 0:c0], 0.0)
                    nc.scalar.activation(out=ex[:, :, c0:], in_=psc[:, :, c0:],
                                         func=EXP, scale=0.125)
                    if m >= 4 * j:  # diagonal tile: multiplicative mask
                        nc.vector.tensor_mul(ex[:], ex[:], masks[c0 // 128][:])
                    if dbg is not None and b == 0 and j == 0 and m == 0:
                        nc.sync.dma_start(out=dbg["ex"][:], in_=ex[:])
                    exq.append((m, ex))

                def emit_av(j=j, mtop=mtop, ctx_ps=ctx_ps):
                    m_av, ex = exq.pop(0)
                    c0 = max(0, 128 * m_av - SC * j)
                    for h in range(PH):
                        nc.tensor.matmul(
                            ctx_ps[h][:, c0:], lhsT=v01[h][:, m_av, :],
                            rhs=ex[:, h, c0:],
                            start=(m_av == 0), stop=(m_av == mtop - 1))

                for m in range(mtop):
                    emit_scores(m)
                    if len(exq) > PIPE:
                        emit_av()
                    if m % 3 == 2 and pj_i[0] < 8:
                        drain_pj(1)  # reserve the last 4 for the bubble
                while exq:
                    emit_av()

                # normalize + scatter to a2a_in (sender-side). GpSimd
                # copies ctx+denom out of PSUM (frees the ctx banks fast);
                # DVE does reciprocal + the normalize-multiply from SBUF.
                x, q4 = j // 2, 4 * (j % 2)
                # both heads' PSUM evacuations first (frees ctx banks fast)
                cus, dns = [], []
                for h in range(PH):
                    cu = cn_p.tile([65, SC], F32, tag="cu",
                                   name=f"cu{b}_{j}_{h}")
                    nc.vector.tensor_copy(cu[:], ctx_ps[h][0:65, :])
                    # reciprocal input must sit at partition 0 (custom DVE
                    # op misreads at a nonzero base partition)
                    dn = rc_p.tile([1, SC], F32, tag="dn", name=f"dn{b}_{j}_{h}")
                    nc.vector.tensor_copy(dn[:], ctx_ps[h][64:65, :])
                    cus.append(cu)
                    dns.append(dn)
                for h in range(PH):
                    cu, dn = cus[h], dns[h]
                    rc = rc_p.tile([1, SC], F32, tag="dn", name=f"rc{b}_{j}_{h}")
                    nc.vector.reciprocal_approx_fast(rc[:], dn[:])
                    rb = rb_p.tile([64, SC], F32, tag="rb")
                    nc.gpsimd.partition_broadcast(rb[:], rc[:])
                    cn = cn_p.tile([64, SC], BF16, tag="cn",
                                   name=f"cn{b}_{j}_{h}")
                    nc.vector.tensor_mul(cn[:], cu[0:64, :], rb[:])
                    if dbg is not None and b == 0 and j == 0 and h == 0:
                        nc.sync.dma_start(out=dbg["cu"][:], in_=cu[:])
                        nc.sync.dma_start(out=dbg["cn"][:], in_=cn[:])
                    hr = slice(64 * h, 64 * (h + 1))
                    for k in range(4):
                        nc.gpsimd.dma_start(
                            out=a2a_in[b][x][q4 + k, hr, :],
                            in_=cn[:, 128 * k:128 * (k + 1)])
                done.add(j)
                if {0, 1} <= done and ("a0" not in done):
                    done.add("a0")
                    emit_a2a(b, 0)
                if {2, 3} <= done and ("a1" not in done):
                    done.add("a1")
                    emit_a2a(b, 1)
                for fn in hooks.get(pos, []):
                    fn()
                if pos == 3:
                    drain_pj(99)  # fill the inter-batch PE bubble

        # ---------- schedule ----------
        # P(0)
        state[("et", 0)] = emit_et_loads(0, split=True)
        groups0, finish0 = start_proj(0)
        for g in groups0:
            g()
        finish0()
        if dbg is not None:
            nc.sync.dma_start(out=dbg["qt"][:], in_=state[("qt", 0)][:])
            nc.sync.dma_start(out=dbg["kt0"][:], in_=state[("kts", 0)][0][:])
            nc.sync.dma_start(out=dbg["v00"][:], in_=state[("v01", 0)][0][:])

        for b in range(B):
            if b + 1 < B:
                state[("et", b + 1)] = emit_et_loads(b + 1)
                pj_groups, pj_finish = start_proj(b + 1)
                state[("pjq", b + 1)] = pj_groups
            else:
                pj_finish = None
                state[("pjq", b + 1)] = []

            hooks = {}
            if b >= 1:
                hooks[0] = [lambda b=b: emit_outproj(b - 1, 0)]
                hooks[1] = [lambda b=b: emit_outproj(b - 1, 1)]
            if b == B - 1:
                # order (0,2,3,1): A2A(3,1) fires after pos2 (j=3) and its
                # transfer overlaps the j=1 m-loop; out-proj(3,1) fills the
                # A2A(3,0) transfer window
                hooks[3] = [lambda: emit_outproj(B - 1, 1)]
            if pj_finish is not None:
                hooks.setdefault(3, []).append(pj_finish)

            emit_attention(b, hooks)

        emit_outproj(B - 1, 0)


_NC_CACHE = None


def _get_nc():
    global _NC_CACHE
    if _NC_CACHE is None:
        _NC_CACHE = build()
    return _NC_CACHE


def kernel(embedded, Wq, Wk, Wv, Wo, bo, _trace=False):
    import ml_dtypes

    bf16 = ml_dtypes.bfloat16
    embedded = np.asarray(embedded, np.float32)
    emb_t = np.ascontiguousarray(embedded.transpose(0, 2, 1)).astype(bf16)
    W = np.stack([np.asarray(Wq), np.asarray(Wk), np.asarray(Wv)]).astype(np.float32)
    wo_t = np.ascontiguousarray(np.asarray(Wo, np.float32).T).astype(
        bf16).reshape(ND, 128, D)
    bo_row = np.asarray(bo, np.float32).reshape(1, D).astype(bf16)

    in_maps = []
    for c in range(NC_):
        w = W[:, 2 * c:2 * c + 2]                  # [3, 2, D, HD]
        w = np.ascontiguousarray(
            w.transpose(0, 2, 1, 3)).reshape(3, ND, 128, 128).astype(bf16)
        in_maps.append({
            "embedded_t": emb_t,
            "w_qkv": w,
            "wo_t": wo_t,
            "bo_row": bo_row,
        })

    nc = _get_nc()
    res = run_bass_kernel_spmd(nc, in_maps, core_ids=list(range(NC_)),
                               trace=_trace)

    if DEBUG:
        import pickle
        dbg_all = [{k: np.asarray(res.results[c]["dbg_" + k]) for k in
                    ("qt", "kt0", "v00", "ex", "cu", "cn", "cat")}
                   for c in range(NC_)]
        with open("/tmp/dbg_stages.pkl", "wb") as f:
            pickle.dump(dbg_all, f)

    out = np.empty((B, S, D), np.float32)
    for c in range(NC_):
        r = res.results[c]["out_shard"]
        for b in range(B):
            for x in range(2):
                t0 = 1024 * x + 128 * c
                out[b, t0:t0 + 128, :] = r[256 * b + 128 * x:
                                           256 * b + 128 * x + 128]
    if _trace:
        return out, res
    return out


# revision 30
# speedup vs baseline: 1.0503x; 1.0503x over previous
"""Multi-headed causal attention on 8 trn2 NeuronCores (Bass/Tile) — v2.

Sharding: tensor-parallel over heads — 2 heads per core, all 4 batches.
v2 redesign vs v1 (463us):
  - bf16 activations/weights end-to-end (host casts embedded/W to bf16):
    halves the 32MB startup DMA and the projection operand traffic.
  - ACT (scalar engine) does ONLY the exp in the attention inner loop;
    causal masking is a precomputed multiplicative bf16 mask applied on
    DVE (was: 1.4us affine_select on GpSimd per diagonal tile).
  - softmax normalization stays on the sender but is restructured:
    DVE dn-copy + fast reciprocal, GpSimd partition_broadcast, fused
    DVE (psum x rb -> bf16) multiply. No ACT involvement.
  - AllToAll split into 8 small per-(batch, seq-half) collectives of
    256KB each, fired mid-attention as soon as their half completes;
    ownership is 128-token stripes per core per batch so every A2A
    carries all 8 destinations. Output projection for each (b,x) block
    is interleaved into the following batch's attention emission.
  - projections of batch b+1 are interleaved m-loop-granular into batch
    b's attention so the PE queue never drains while ACT paces exp.
  - PE warmup matmuls during the initial DMA fill (HAM un-throttle).
  - output-projection bias folded into a K=1 matmul; out rows are
    (batch, half, stripe) blocks reassembled on host.
"""
import sys

sys.path.insert(0, "/opt/trn_rl_repo")

import numpy as np

import concourse.bass as bass
import concourse.tile as tile
from concourse import bacc, mybir
from concourse.bass_utils import run_bass_kernel_spmd

B, S, D, H, HD = 4, 2048, 1024, 16, 64
NC_ = 8          # cores
PH = 2           # heads per core
SC = 512         # s_q chunk
NK = S // 128    # 16 s_k chunks of 128
ND = D // 128    # 8 contraction chunks of 128
DEBUG = False
F32 = mybir.dt.float32
BF16 = mybir.dt.bfloat16
EXP = mybir.ActivationFunctionType.Exp
GE = mybir.AluOpType.is_ge


def build():
    nc = bacc.Bacc("TRN2", target_bir_lowering=False, debug=False, num_devices=NC_)

    emb_t = nc.dram_tensor("embedded_t", [B, D, S], BF16, kind="ExternalInput").ap()
    w_qkv = nc.dram_tensor("w_qkv", [3, ND, 128, 128], BF16, kind="ExternalInput").ap()
    wo_t = nc.dram_tensor("wo_t", [ND, 128, D], BF16, kind="ExternalInput").ap()
    bo_row = nc.dram_tensor("bo_row", [1, D], BF16, kind="ExternalInput").ap()
    out_shard = nc.dram_tensor("out_shard", [1024, D], F32, kind="ExternalOutput").ap()
    dbg = None
    if DEBUG:
        dbg = {
            "qt": nc.dram_tensor("dbg_qt", [128, S], BF16, kind="ExternalOutput").ap(),
            "kt0": nc.dram_tensor("dbg_kt0", [128, S], BF16, kind="ExternalOutput").ap(),
            "v00": nc.dram_tensor("dbg_v00", [128, NK, 128], BF16, kind="ExternalOutput").ap(),
            "ex": nc.dram_tensor("dbg_ex", [128, 2, SC], BF16, kind="ExternalOutput").ap(),
            "cu": nc.dram_tensor("dbg_cu", [65, SC], F32, kind="ExternalOutput").ap(),
            "cn": nc.dram_tensor("dbg_cn", [64, SC], BF16, kind="ExternalOutput").ap(),
            "cat": nc.dram_tensor("dbg_cat", [NC_, 128, 128], BF16, kind="ExternalOutput").ap(),
        }

    with tile.TileContext(nc) as tc:
        _build_body(nc, tc, emb_t, w_qkv, wo_t, bo_row, out_shard, dbg)

    nc.compile()
    return nc


def _build_body(nc, tc, emb_t, w_qkv, wo_t, bo_row, out_shard, dbg=None):
    from contextlib import ExitStack

    ctx = ExitStack()
    with ctx:
        const = ctx.enter_context(tc.tile_pool(name="const", bufs=1))
        # PSUM: scores 2x[128,1024](4 banks) + ctx 2x[128,512](2) +
        # misc 2x[128,512](2) = 8 banks
        ps_sc = ctx.enter_context(tc.tile_pool(name="ps_sc", bufs=2, space="PSUM"))
        ps_ctx = ctx.enter_context(tc.tile_pool(name="ps_ctx", bufs=2, space="PSUM"))
        ps_ms = ctx.enter_context(tc.tile_pool(name="ps_ms", bufs=2, space="PSUM"))
        dram = ctx.enter_context(tc.tile_pool(name="dram", bufs=1, space="DRAM"))

        etp = ctx.enter_context(tc.tile_pool(name="etp", bufs=16))
        qtp = ctx.enter_context(tc.tile_pool(name="qtp", bufs=2))
        ktp = ctx.enter_context(tc.tile_pool(name="ktp", bufs=2))
        vtp = ctx.enter_context(tc.tile_pool(name="vtp", bufs=2))
        vsb = ctx.enter_context(tc.tile_pool(name="vsb", bufs=2))
        exp_p = ctx.enter_context(tc.tile_pool(name="exp_p", bufs=4))
        cn_p = ctx.enter_context(tc.tile_pool(name="cn_p", bufs=4))
        rc_p = ctx.enter_context(tc.tile_pool(name="rc_p", bufs=4))
        rb_p = ctx.enter_context(tc.tile_pool(name="rb_p", bufs=2))
        cat_p = ctx.enter_context(tc.tile_pool(name="cat_p", bufs=16))
        ob_p = ctx.enter_context(tc.tile_pool(name="ob_p", bufs=2))

        # ---- startup DMAs (SP queue): weights, batch-0 activations ----
        wq_all = const.tile([128, 24, 128], BF16, tag="wq_all")
        for p in range(3):
            nc.sync.dma_start(out=wq_all[:, 8 * p:8 * (p + 1), :],
                              in_=bass.AP(
                tensor=w_qkv.tensor, offset=131072 * p,
                ap=[[128, 128], [16384, 8], [1, 128]]))
        wq_sb = [[wq_all[:, 8 * p + c, :] for c in range(ND)] for p in range(3)]

        wot_sb = [const.tile([128, D], BF16, tag=f"wo{c}", name=f"wo{c}")
                  for c in range(ND)]
        for c in range(ND):
            nc.sync.dma_start(out=wot_sb[c][:], in_=wo_t[c])
        bo_sb = const.tile([1, D], BF16, tag="bo1")
        nc.sync.dma_start(out=bo_sb[:], in_=bo_row[:])

        # ---- PE warmup first: its memset must not queue behind the
        # slow gpsimd mask builds below ----
        wu_src = const.tile([128, SC], BF16, tag="wu")
        nc.gpsimd.memset(wu_src[:], 0.0)
        wu_ps = ps_ctx.tile([128, SC], F32, tag="ctx", name="wu_ps")
        for _ in range(22):
            nc.tensor.matmul(wu_ps[:], lhsT=wu_src[:, 0:128], rhs=wu_src[:],
                             start=True, stop=True)

        # ---- constants ----
        ones_bf = const.tile([1, 128], BF16, tag="ones_bf")
        nc.gpsimd.memset(ones_bf[:], 1.0)
        ones_r = const.tile([128, 1], BF16, tag="ones_r")
        nc.gpsimd.memset(ones_r[:], 1.0)

        # causal masks for diagonal tiles: keep col c (mod 512) >= p + 128*mi
        masks = []
        for mi in range(4):
            mk = const.tile([128, 2, SC], BF16, tag=f"mask{mi}", name=f"mask{mi}")
            nc.gpsimd.memset(mk[:], 1.0)
            nc.gpsimd.affine_select(
                out=mk[:], in_=mk[:], compare_op=GE, fill=0.0,
                base=-128 * mi, pattern=[[0, 2], [1, SC]],
                channel_multiplier=-1)
            masks.append(mk)

        # a2a buffers: one pair per (batch, seq-half); slot o = 128-token
        # stripe for core o, rows = this core's 128 ctx dims (normalized)
        a2a_in = [[dram.tile([NC_, 128, 128], BF16, tag=f"a2a_in{b}_{x}",
                             name=f"a2a_in{b}_{x}") for x in range(2)]
                  for b in range(B)]
        a2a_out = [[dram.tile([NC_, 128, 128], BF16, tag=f"a2a_out{b}_{x}",
                              name=f"a2a_out{b}_{x}")
                    for x in range(2)] for b in range(B)]

        def emit_a2a(b, x):
            nc.gpsimd.collective_compute(
                "AllToAll", mybir.AluOpType.bypass,
                replica_groups=[list(range(NC_))],
                ins=[a2a_in[b][x].opt()], outs=[a2a_out[b][x].opt()])

        # ---------- phase builders ----------
        state = {}

        def emit_et_loads(b, split=False):
            et = {c: etp.tile([128, S], BF16, tag="et", name=f"et{b}_{c}")
                  for c in range(ND)}
            if split:
                # quartered, two queues: the first pj-group can start after
                # only the j4=0 column block has landed
                for j4 in range(4):
                    for c in range(ND):
                        eng = nc.sync if (c % 2 == 0) else nc.scalar
                        eng.dma_start(
                            out=et[c][:, SC * j4:SC * (j4 + 1)],
                            in_=emb_t[b, 128 * c:128 * (c + 1),
                                      SC * j4:SC * (j4 + 1)])
            else:
                # ACT queue: keeps SP free for cat loads (out-proj path)
                for c in range(ND):
                    nc.scalar.dma_start(
                        out=et[c][:], in_=emb_t[b, 128 * c:128 * (c + 1), :])
            return et

        def start_proj(b):
            """Allocate batch-b projection outputs + zero-pads; return the
            list of 12 pj-group closures and the finish closure."""
            qt = qtp.tile([128, S], BF16, tag="qt", name=f"qt{b}")
            kt0 = ktp.tile([128, S], BF16, tag="kt0", name=f"kt0_{b}")
            kt1 = ktp.tile([128, S], BF16, tag="kt1", name=f"kt1_{b}")
            nc.vector.memset(kt0[64:128, :], 0.0)
            nc.vector.memset(kt1[0:64, :], 0.0)
            vt = vtp.tile([128, S], BF16, tag="vt", name=f"vt{b}")
            groups = []
            # emission order: vt groups first (transposes need them), then
            # qt/kt for j4 0-1 (needed by next batch's j=0/1), then the rest
            # which fill the inter-batch PE bubble
            order = ([(j4, 2) for j4 in range(4)]
                     + [(j4, p) for j4 in (0, 1) for p in (0, 1)]
                     + [(j4, p) for j4 in (2, 3) for p in (0, 1)])
            for j4, p in order:
                if True:
                    def g(j4=j4, p=p):
                        sl = slice(SC * j4, SC * (j4 + 1))
                        ps = ps_ms.tile([128, SC], F32, tag="ms",
                                        name=f"pj{b}_{j4}_{p}")
                        for c in range(ND):
                            rhs = state[("et", b)][c][:, sl]
                            nc.tensor.matmul(
                                ps[:], lhsT=wq_sb[p][c], rhs=rhs,
                                start=(c == 0), stop=(c == ND - 1))
                        if p == 0:
                            nc.vector.tensor_copy(qt[:, sl], ps[:])
                        elif p == 1:
                            nc.vector.tensor_copy(kt0[0:64, sl], ps[0:64, :])
                            nc.vector.tensor_copy(kt1[64:128, sl], ps[64:128, :])
                        else:
                            nc.scalar.copy(vt[:, sl], ps[:])
                    groups.append(g)

            def finish():
                # V natural layout padded to 128 cols: V | ones | zeros.
                # XBAR transpose-DMA: v01[h][p, k, c] = vt[64h + c, 128k + p]
                v01 = [vsb.tile([128, NK, 128], BF16, tag=f"v{h}",
                                name=f"v{h}_{b}") for h in range(PH)]
                for h in range(PH):
                    nc.vector.memset(v01[h][:, :, 65:128], 0.0)
                    nc.vector.memset(v01[h][:, :, 64:65], 1.0)
                    nc.sync.dma_start_transpose(
                        out=v01[h][:, :, 0:64], in_=vt[64 * h:64 * (h + 1), :])
                state[("v01", b)] = v01

            state[("qt", b)] = qt
            state[("kts", b)] = [kt0, kt1]
            return groups, finish

        def emit_outproj(b, x):
            """cat loads + output projection for this core's 128-token
            stripe of (batch b, half x)."""
            cats = []
            for r in range(NC_):
                ct = cat_p.tile([128, 128], BF16, tag="cat",
                                name=f"cat{b}_{x}_{r}")
                nc.sync.dma_start(out=ct[:], in_=a2a_out[b][x][r])
                if dbg is not None and b == 0 and x == 0:
                    nc.sync.dma_start(out=dbg["cat"][r], in_=ct[:])
                cats.append(ct)
            row0 = 256 * b + 128 * x
            for n in range(2):
                po = ps_ms.tile([128, SC], F32, tag="ms", name=f"po{b}_{x}_{n}")
                for kp in range(ND):
                    nc.tensor.matmul(
                        po[:], lhsT=cats[kp][:],
                        rhs=wot_sb[kp][:, SC * n:SC * (n + 1)],
                        start=(kp == 0), stop=False)
                nc.tensor.matmul(
                    po[:], lhsT=ones_bf[:],
                    rhs=bo_sb[:, SC * n:SC * (n + 1)],
                    start=False, stop=True)
                ob = ob_p.tile([128, SC], F32, tag="ob")
                nc.scalar.copy(ob[:], po[:])
                nc.gpsimd.dma_start(
                    out=out_shard[row0:row0 + 128, SC * n:SC * (n + 1)],
                    in_=ob[:])

        def emit_attention(b, hooks):
            """hooks: dict j -> list of closures emitted after chunk j's
            m-loop (projection groups of b+1, out-proj blocks, ...)."""
            qt = state[("qt", b)]
            kts = state[("kts", b)]
            v01 = state[("v01", b)]
            pj_queue = state.get(("pjq", b + 1), [])
            pj_i = [0]

            def drain_pj(n):
                for _ in range(n):
                    if pj_i[0] < len(pj_queue):
                        pj_queue[pj_i[0]]()
                        pj_i[0] += 1

            # last batch: (0,2,3,1) so the final A2A's transfer overlaps
            # the j=1 m-loop and out-proj(3,1)
            j_order = (0, 2, 3, 1) if b == B - 1 else (0, 1, 2, 3)
            done = set()
            for pos, j in enumerate(j_order):
                mtop = 4 * j + 4
                ctx_ps = [ps_ctx.tile([128, SC], F32, tag="ctx",
                                      name=f"ctx{b}_{j}_{h}")
                          for h in range(PH)]
                PIPE = 1
                exq = []

                def emit_scores(m, j=j, ctx_ps=ctx_ps):
                    c0 = max(0, 128 * m - SC * j)
                    psc = ps_sc.tile([128, 2, SC], F32, tag="sc",
                                     name=f"sc{b}_{j}_{m}")
                    for h in range(PH):
                        nc.tensor.matmul(
                            psc[:, h, c0:],
                            lhsT=kts[h][:, 128 * m:128 * (m + 1)],
                            rhs=qt[:, SC * j + c0:SC * (j + 1)],
                            start=True, stop=True)
                    ex = exp_p.tile([128, 2, SC], BF16, tag="ex",
                                    name=f"ex{b}_{j}_{m}")
                    if c0 > 0:
                        # zero the never-exp'd leading zones so the mask
                        # multiply can't hit stale inf/NaN
                        nc.vector.memset(ex[:, :, 0:c0], 0.0)
                    nc.scalar.activation(out=ex[:, :, c0:], in_=psc[:, :, c0:],
                                         func=EXP, scale=0.125)
                    if m >= 4 * j:  # diagonal tile: multiplicative mask
                        nc.vector.tensor_mul(ex[:], ex[:], masks[c0 // 128][:])
                    if dbg is not None and b == 0 and j == 0 and m == 0:
                        nc.sync.dma_start(out=dbg["ex"][:], in_=ex[:])
                    exq.append((m, ex))

                def emit_av(j=j, mtop=mtop, ctx_ps=ctx_ps):
                    m_av, ex = exq.pop(0)
                    c0 = max(0, 128 * m_av - SC * j)
                    for h in range(PH):
                        nc.tensor.matmul(
                            ctx_ps[h][:, c0:], lhsT=v01[h][:, m_av, :],
                            rhs=ex[:, h, c0:],
                            start=(m_av == 0), stop=(m_av == mtop - 1))

                for m in range(mtop):
                    emit_scores(m)
                    if len(exq) > PIPE:
                        emit_av()
                    if m % 3 == 2 and pj_i[0] < 8:
                        drain_pj(1)  # reserve the last 4 for the bubble
                while exq:
                    emit_av()

                # normalize + scatter to a2a_in (sender-side). GpSimd
                # copies ctx+denom out of PSUM (frees the ctx banks fast);
                # DVE does reciprocal + the normalize-multiply from SBUF.
                x, q4 = j // 2, 4 * (j % 2)
                # both heads' PSUM evacuations first (frees ctx banks fast)
                cus, dns = [], []
                for h in range(PH):
                    cu = cn_p.tile([65, SC], F32, tag="cu",
                                   name=f"cu{b}_{j}_{h}")
                    nc.vector.tensor_copy(cu[:], ctx_ps[h][0:65, :])
                    # reciprocal input must sit at partition 0 (custom DVE
                    # op misreads at a nonzero base partition)
                    dn = rc_p.tile([1, SC], F32, tag="dn", name=f"dn{b}_{j}_{h}")
                    nc.vector.tensor_copy(dn[:], ctx_ps[h][64:65, :])
                    cus.append(cu)
                    dns.append(dn)
                for h in range(PH):
                    cu, dn = cus[h], dns[h]
                    rc = rc_p.tile([1, SC], F32, tag="dn", name=f"rc{b}_{j}_{h}")
                    nc.vector.reciprocal_approx_fast(rc[:], dn[:])
                    rb = rb_p.tile([64, SC], F32, tag="rb")
                    nc.gpsimd.partition_broadcast(rb[:], rc[:])
                    cn = cn_p.tile([64, SC], BF16, tag="cn",
                                   name=f"cn{b}_{j}_{h}")
                    nc.vector.tensor_mul(cn[:], cu[0:64, :], rb[:])
                    if dbg is not None and b == 0 and j == 0 and h == 0:
                        nc.sync.dma_start(out=dbg["cu"][:], in_=cu[:])
                        nc.sync.dma_start(out=dbg["cn"][:], in_=cn[:])
                    hr = slice(64 * h, 64 * (h + 1))
                    for k in range(4):
                        nc.gpsimd.dma_start(
                            out=a2a_in[b][x][q4 + k, hr, :],
                            in_=cn[:, 128 * k:128 * (k + 1)])
                done.add(j)
                if {0, 1} <= done and ("a0" not in done):
                    done.add("a0")
                    emit_a2a(b, 0)
                if {2, 3} <= done and ("a1" not in done):
                    done.add("a1")
                    emit_a2a(b, 1)
                for fn in hooks.get(pos, []):
                    fn()
                if pos == 3:
                    drain_pj(99)  # fill the inter-batch PE bubble

        # ---------- schedule ----------
        # P(0)
        state[("et", 0)] = emit_et_loads(0, split=True)
        groups0, finish0 = start_proj(0)
        for g in groups0:
            g()
        finish0()
        if dbg is not None:
            nc.sync.dma_start(out=dbg["qt"][:], in_=state[("qt", 0)][:])
            nc.sync.dma_start(out=dbg["kt0"][:], in_=state[("kts", 0)][0][:])
            nc.sync.dma_start(out=dbg["v00"][:], in_=state[("v01", 0)][0][:])

        for b in range(B):
            if b + 1 < B:
                state[("et", b + 1)] = emit_et_loads(b + 1)
                pj_groups, pj_finish = start_proj(b + 1)
                state[("pjq", b + 1)] = pj_groups
            else:
                pj_finish = None
                state[("pjq", b + 1)] = []

            hooks = {}
            if b >= 1:
                hooks[0] = [lambda b=b: emit_outproj(b - 1, 0)]
                hooks[1] = [lambda b=b: emit_outproj(b - 1, 1)]
            if b == B - 1:
                # order (0,2,3,1): A2A(3,1) fires after pos2 (j=3) and its
                # transfer overlaps the j=1 m-loop; out-proj(3,1) fills the
                # A2A(3,0) transfer window
                hooks[3] = [lambda: emit_outproj(B - 1, 1)]
            if pj_finish is not None:
                hooks.setdefault(3, []).append(pj_finish)

            emit_attention(b, hooks)

        emit_outproj(B - 1, 0)


_NC_CACHE = None


def _get_nc():
    global _NC_CACHE
    if _NC_CACHE is None:
        _NC_CACHE = build()
    return _NC_CACHE


def kernel(embedded, Wq, Wk, Wv, Wo, bo, _trace=False):
    import ml_dtypes

    bf16 = ml_dtypes.bfloat16
    embedded = np.asarray(embedded, np.float32)
    emb_t = np.ascontiguousarray(embedded.transpose(0, 2, 1)).astype(bf16)
    W = np.stack([np.asarray(Wq), np.asarray(Wk), np.asarray(Wv)]).astype(np.float32)
    wo_t = np.ascontiguousarray(np.asarray(Wo, np.float32).T).astype(
        bf16).reshape(ND, 128, D)
    bo_row = np.asarray(bo, np.float32).reshape(1, D).astype(bf16)

    in_maps = []
    for c in range(NC_):
        w = W[:, 2 * c:2 * c + 2]                  # [3, 2, D, HD]
        w = np.ascontiguousarray(
            w.transpose(0, 2, 1, 3)).reshape(3, ND, 128, 128).astype(bf16)
        in_maps.append({
            "embedded_t": emb_t,
            "w_qkv": w,
            "wo_t": wo_t,
            "bo_row": bo_row,
        })

    nc = _get_nc()
    res = run_bass_kernel_spmd(nc, in_maps, core_ids=list(range(NC_)),
                               trace=_trace)

    if DEBUG:
        import pickle
        dbg_all = [{k: np.asarray(res.results[c]["dbg_" + k]) for k in
                    ("qt", "kt0", "v00", "ex", "cu", "cn", "cat")}
                   for c in range(NC_)]
        with open("/tmp/dbg_stages.pkl", "wb") as f:
            pickle.dump(dbg_all, f)

    out = np.empty((B, S, D), np.float32)
    for c in range(NC_):
        r = res.results[c]["out_shard"]
        for b in range(B):
            for x in range(2):
                t0 = 1024 * x + 128 * c
                out[b, t0:t0 + 128, :] = r[256 * b + 128 * x:
                                           256 * b + 128 * x + 128]
    if _trace:
        return out, res
    return out
